# revision 12
# baseline (speedup 1.0000x reference)
"""Trainium2 Bass kernel for the custom attention module.

Self-contained: hardcodes shapes B=16, S=626, D=768, H=12, HD=64.
Shards batch over 8 NeuronCores (2 batches/core), no collectives.

Outputs (matching the reference): (attention_output, attention_probs, contribution)
"""

import numpy as np

B, S, D, H, HD = 16, 626, 768, 12, 64
NCORES = 8
BL = B // NCORES  # batches per core

# token tiles of 626: (start, size)
QTS = [(0, 128), (128, 128), (256, 128), (384, 128), (512, 114)]
N626 = [(0, 512), (512, 114)]   # free-dim chunks for 626-wide matmul outputs
N768 = [(0, 512), (512, 256)]   # free-dim chunks for 768-wide matmul outputs

_CACHE = {}


def _build():
    import concourse.bass as bass
    import concourse.mybir as mybir
    from concourse import bacc
    from concourse.tile import TileContext
    from concourse.masks import make_identity
    from concourse.bass import ds
    from contextlib import ExitStack

    F32 = mybir.dt.float32
    BF = mybir.dt.bfloat16
    I32 = mybir.dt.int32
    AX = mybir.AxisListType.X
    ALU = mybir.AluOpType
    AF = mybir.ActivationFunctionType

    nc = bacc.Bacc()

    hid = nc.declare_dram_parameter("hidden_states", [BL, S, D], F32, isOutput=False)
    msk = nc.declare_dram_parameter("mask", [BL, S - 1], I32, isOutput=False)
    wq_d = nc.declare_dram_parameter("Wq", [D, D], F32, isOutput=False)
    bq_d = nc.declare_dram_parameter("bq", [D], F32, isOutput=False)
    wk_d = nc.declare_dram_parameter("Wk", [D, D], F32, isOutput=False)
    bk_d = nc.declare_dram_parameter("bk", [D], F32, isOutput=False)
    wv_d = nc.declare_dram_parameter("Wv", [D, D], F32, isOutput=False)
    bv_d = nc.declare_dram_parameter("bv", [D], F32, isOutput=False)
    wo_d = nc.declare_dram_parameter("Wo", [D, D], F32, isOutput=False)
    bo_d = nc.declare_dram_parameter("bo", [D], F32, isOutput=False)

    o_out = nc.declare_dram_parameter("out", [BL, S, D], F32, isOutput=True)
    o_prb = nc.declare_dram_parameter("probs", [BL, H, S, S], F32, isOutput=True)
    o_ctr = nc.declare_dram_parameter("contrib", [BL, H, S], F32, isOutput=True)

    with ExitStack() as ctx:
        tc = ctx.enter_context(TileContext(nc))
        singles = ctx.enter_context(tc.tile_pool(name="singles", bufs=1))
        pool = ctx.enter_context(tc.tile_pool(name="work", bufs=2))
        pp_mm = ctx.enter_context(tc.tile_pool(name="psmm", bufs=2, space="PSUM"))
        pp_tp = ctx.enter_context(tc.tile_pool(name="pstp", bufs=2, space="PSUM"))
        pp_ac = ctx.enter_context(tc.tile_pool(name="psac", bufs=1, space="PSUM"))

        # ---------- one-time setup ----------
        ident = singles.tile([128, 128], BF, tag="ident")
        make_identity(nc, ident)
        ones_bf = singles.tile([1, 128], BF, tag="ones")
        nc.vector.memset(ones_bf, 1.0)

        # weights, cast to bf16 during DMA (SWDGE).  lhsT layout [c=d_in, d_out]
        w_sb = {}
        for name, wd in (("q", wq_d), ("k", wk_d), ("v", wv_d), ("o", wo_d)):
            w_sb[name] = []
            for j in range(6):
                t = singles.tile([128, D], BF, tag=f"w{name}{j}")
                nc.gpsimd.dma_start(out=t, in_=wd[ds(j * 128, 128), :])
                w_sb[name].append(t)

        # per-partition bias chunks for QT/KT copyback (ACT bias operand, f32)
        bq_sb, bk_sb = [], []
        for bname, bd, lst in (("bq", bq_d, bq_sb), ("bk", bk_d, bk_sb)):
            bv_ap = bd[:].rearrange("(p o) -> p o", o=1)  # [768, 1]
            for j in range(6):
                t = singles.tile([128, 1], F32, tag=f"{bname}{j}")
                nc.sync.dma_start(out=t, in_=bv_ap[ds(j * 128, 128), :])
                lst.append(t)
        # bv as a bf16 row for the bias-row matmul trick
        bv_bf = singles.tile([1, D], BF, tag="bvbf")
        nc.gpsimd.dma_start(out=bv_bf, in_=bv_d[:].rearrange("(o d) -> o d", o=1))
        # bo broadcast to all 128 partitions (for the output epilogue add)
        bo_bc = singles.tile([128, D], F32, tag="bobc")
        bo_row = bo_d[:].rearrange("(o d) -> o d", o=1)  # [1, 768]
        nc.sync.dma_start(out=bo_bc, in_=bo_row.to_broadcast((128, D)))

        # ---------- per-batch ----------
        for b in range(BL):
            # boost row: 0.25 where mask626 == 0 (col 0 is the prepended CLS zero)
            mask_sb = pool.tile([1, S - 1], I32, tag="msk")
            nc.sync.dma_start(out=mask_sb, in_=msk[b].rearrange("(o k) -> o k", o=1))
            boost = pool.tile([1, S], F32, tag="boost")
            nc.vector.memset(boost[0:1, 0:1], 0.25)
            nc.vector.tensor_scalar(
                out=boost[0:1, 1:S], in0=mask_sb, scalar1=0, scalar2=0.25,
                op0=ALU.is_equal, op1=ALU.mult,
            )

            # hidden load (cast to bf16) + PE transpose -> hT [768, 626]
            hid_bf = []
            for (ts_, tz) in QTS:
                t = pool.tile([128, D], BF, tag="hidbf", bufs=5)
                nc.gpsimd.dma_start(out=t[0:tz, :], in_=hid[b, ds(ts_, tz), :])
                hid_bf.append(t)
            hT = []
            for j in range(6):
                ps = pp_tp.tile([128, S], BF, tag="tp")
                for ti, (ts_, tz) in enumerate(QTS):
                    nc.tensor.transpose(
                        out=ps[0:128, ds(ts_, tz)],
                        in_=hid_bf[ti][0:tz, ds(j * 128, 128)],
                        identity=ident[0:tz, 0:tz],
                    )
                t = pool.tile([128, S], BF, tag="hT", bufs=6)
                nc.vector.tensor_copy(out=t, in_=ps)
                hT.append(t)

            # QKV projections
            QT_sb, KT_sb = [], []
            for wname, blst, outlst in (("q", bq_sb, QT_sb), ("k", bk_sb, KT_sb)):
                for j in range(6):
                    ps = pp_mm.tile([128, S], F32, tag="mm")
                    for (ns, nz) in N626:
                        for c in range(6):
                            nc.tensor.matmul(
                                out=ps[:, ds(ns, nz)],
                                lhsT=w_sb[wname][c][:, ds(j * 128, 128)],
                                rhs=hT[c][:, ds(ns, nz)],
                                start=(c == 0), stop=(c == 5),
                            )
                    t = pool.tile([128, S], BF, tag=f"{wname}T", bufs=6)
                    nc.vector.tensor_scalar(
                        out=t, in0=ps, scalar1=blst[j][0:128, 0:1], scalar2=None,
                        op0=ALU.add,
                    )
                    outlst.append(t)
            V_sb = []
            for (ts_, tz) in QTS:
                ps = pp_mm.tile([128, D], F32, tag="mm")
                for (ns, nz) in N768:
                    for c in range(6):
                        nc.tensor.matmul(
                            out=ps[0:tz, ds(ns, nz)],
                            lhsT=hT[c][:, ds(ts_, tz)],
                            rhs=w_sb["v"][c][:, ds(ns, nz)],
                            start=(c == 0), stop=False,
                        )
                    nc.tensor.matmul(
                        out=ps[0:tz, ds(ns, nz)],
                        lhsT=ones_bf[0:1, 0:tz],
                        rhs=bv_bf[0:1, ds(ns, nz)],
                        start=False, stop=True,
                    )
                t = pool.tile([128, D], BF, tag="V", bufs=5)
                nc.vector.tensor_copy(out=t[0:tz, :], in_=ps[0:tz, :])
                V_sb.append(t)

            craw = pool.tile([H, S], F32, tag="craw")
            ctxT_sb = []

            # ---------- head pairs ----------
            for p in range(H // 2):
                rmax = {}
                c0row = {}
                # col0 mini-matmuls: scores[:, 0] as a row over q (for contribution)
                for m in (0, 1):
                    h = 2 * p + m
                    c0 = pp_ac.tile([1, S], F32, tag="acc")
                    for (ns, nz) in N626:
                        nc.tensor.matmul(
                            out=c0[0:1, ds(ns, nz)],
                            lhsT=KT_sb[p][ds(64 * m, 64), 0:1],
                            rhs=QT_sb[p][ds(64 * m, 64), ds(ns, nz)],
                            start=True, stop=True,
                        )
                    t0 = pool.tile([1, S], F32, tag="c0row", bufs=3)
                    nc.vector.tensor_copy(out=t0, in_=c0[0:1, :])
                    c0row[m] = t0

                norm_sb = {}
                for m in (0, 1):
                    nrm = pool.tile([128, 5 * S], BF, tag=f"norm{m}", name=f"nrm{m}")
                    norm_sb[m] = nrm
                # interleave heads A/B so their K=64 matmuls land in different
                # PE row groups (tile_position from base_partition) and overlap
                for qi, (qs, qz) in enumerate(QTS):
                    for m in (0, 1):
                        h = 2 * p + m
                        nrm = norm_sb[m]
                        sps = pp_mm.tile([128, S], F32, tag="mm")
                        for (ns, nz) in N626:
                            nc.tensor.matmul(
                                out=sps[0:qz, ds(ns, nz)],
                                lhsT=QT_sb[p][ds(64 * m, 64), ds(qs, qz)],
                                rhs=KT_sb[p][ds(64 * m, 64), ds(ns, nz)],
                                start=True, stop=True,
                            )
                        r0c = None
                        if qi == 0:
                            # CLS-row boost, done on an SBUF copy so that DVE
                            # never reads the scores PSUM (keeps matmul waits <= 2)
                            r0c = pool.tile([1, S], F32, tag="r0c", bufs=2)
                            nc.scalar.copy(out=r0c, in_=sps[0:1, :])
                            rx = pool.tile([1, 1], F32, tag="rmax", bufs=4)
                            nc.vector.reduce_max(out=rx, in_=r0c, axis=AX)
                            rmax[m] = rx
                            btmp = pool.tile([1, S], F32, tag="btmp")
                            nc.vector.tensor_scalar(
                                out=btmp, in0=boost, scalar1=rx[0:1, 0:1],
                                scalar2=None, op0=ALU.mult,
                            )
                            nc.vector.tensor_tensor(
                                out=r0c, in0=r0c, in1=btmp, op=ALU.add,
                            )
                            # contribution element q=0 gets the same boost (mask626[0]=0)
                            ctmp = pool.tile([1, 1], F32, tag="ctmp")
                            nc.vector.tensor_scalar(
                                out=ctmp, in0=rx, scalar1=0.25, scalar2=None,
                                op0=ALU.mult,
                            )
                            nc.vector.tensor_tensor(
                                out=c0row[m][0:1, 0:1], in0=c0row[m][0:1, 0:1],
                                in1=ctmp[0:1, 0:1], op=ALU.add,
                            )
                            # assemble the fixed row into the packed per-head table
                            nc.sync.dma_start(
                                out=craw[h:h + 1, :], in_=c0row[m],
                            )
                        ex = pool.tile([128, S], BF, tag="exp", bufs=4)
                        sm = pool.tile([128, 1], F32, tag="sums", bufs=8)
                        nc.scalar.activation(
                            out=ex[0:qz, :], in_=sps[0:qz, :], func=AF.Exp,
                            bias=0.0, scale=0.125, accum_out=sm[0:qz, 0:1],
                        )
                        if qi == 0:
                            # overwrite row 0 with the boosted version (+ its sum)
                            nc.scalar.activation(
                                out=ex[0:1, :], in_=r0c, func=AF.Exp,
                                bias=0.0, scale=0.125, accum_out=sm[0:1, 0:1],
                            )
                        rc = pool.tile([128, 1], F32, tag="rcp", bufs=8)
                        nc.vector.reciprocal(out=rc[0:qz, :], in_=sm[0:qz, :])
                        nc.vector.tensor_scalar(
                            out=nrm[0:qz, ds(qi * S, S)], in0=ex[0:qz, :],
                            scalar1=rc[0:qz, 0:1], scalar2=None, op0=ALU.mult,
                        )
                # fp32 probs output via casting SWDGE DMA (bf16 -> f32)
                for m in (0, 1):
                    h = 2 * p + m
                    nrm = norm_sb[m]
                    nc.gpsimd.dma_start(
                        out=o_prb[b, h, 0:512, :].rearrange("(t p) k -> p t k", p=128),
                        in_=nrm[0:128, 0:4 * S].rearrange("p (t k) -> p t k", k=S),
                    )
                    nc.gpsimd.dma_start(
                        out=o_prb[b, h, 512:S, :],
                        in_=nrm[0:114, ds(4 * S, S)],
                    )

                # transpose probs + context matmul (accumulated over k tiles)
                cps = pp_ac.tile([128, S], F32, tag="acc")
                for ki, (ks, kz) in enumerate(QTS):
                    for m in (0, 1):
                        h = 2 * p + m
                        eps = pp_tp.tile([128, S], BF, tag="tp")
                        for qi, (qs, qz) in enumerate(QTS):
                            nc.tensor.transpose(
                                out=eps[0:kz, ds(qs, qz)],
                                in_=norm_sb[m][0:qz, ds(qi * S + ks, kz)],
                                identity=ident[0:qz, 0:qz],
                            )
                        eT = pool.tile([128, S], BF, tag="eT", bufs=3)
                        nc.vector.tensor_copy(out=eT[0:kz, :], in_=eps[0:kz, :])
                        for (ns, nz) in N626:
                            nc.tensor.matmul(
                                out=cps[ds(64 * m, 64), ds(ns, nz)],
                                lhsT=V_sb[ki][0:kz, ds(64 * h, 64)],
                                rhs=eT[0:kz, ds(ns, nz)],
                                start=(ki == 0), stop=(ki == 4),
                            )
                ct = pool.tile([128, S], BF, tag="ctxT", bufs=6)
                nc.vector.tensor_copy(out=ct, in_=cps)
                ctxT_sb.append(ct)

            # contribution: softmax over q of scores[:, 0] per head
            ce = pool.tile([H, S], F32, tag="ce")
            cs = pool.tile([H, 1], F32, tag="cs")
            nc.scalar.activation(
                out=ce, in_=craw, func=AF.Exp, bias=0.0, scale=0.125,
                accum_out=cs[0:H, 0:1],
            )
            cr = pool.tile([H, 1], F32, tag="cr")
            nc.vector.reciprocal(out=cr, in_=cs)
            co = pool.tile([H, S], F32, tag="co")
            nc.vector.tensor_scalar(
                out=co, in0=ce, scalar1=cr[0:H, 0:1], scalar2=None, op0=ALU.mult,
            )
            nc.sync.dma_start(out=o_ctr[b], in_=co)

            # output projection + bias
            for (qs, qz) in QTS:
                ops = pp_mm.tile([128, D], F32, tag="mm")
                for (ns, nz) in N768:
                    for c in range(6):
                        nc.tensor.matmul(
                            out=ops[0:qz, ds(ns, nz)],
                            lhsT=ctxT_sb[c][:, ds(qs, qz)],
                            rhs=w_sb["o"][c][:, ds(ns, nz)],
                            start=(c == 0), stop=(c == 5),
                        )
                osb = pool.tile([128, D], F32, tag="osb")
                nc.vector.tensor_tensor(
                    out=osb[0:qz, :], in0=ops[0:qz, :], in1=bo_bc[0:qz, :],
                    op=ALU.add,
                )
                nc.sync.dma_start(out=o_out[b, ds(qs, qz), :], in_=osb[0:qz, :])

    nc.compile()
    return nc


def _in_maps(inputs):
    f32 = lambda x: np.ascontiguousarray(np.asarray(x), dtype=np.float32)
    maps = []
    for i in range(NCORES):
        sl = slice(BL * i, BL * (i + 1))
        maps.append({
            "hidden_states": f32(inputs["hidden_states"][sl]),
            "mask": np.ascontiguousarray(np.asarray(inputs["mask"][sl]), dtype=np.int32),
            "Wq": f32(inputs["Wq"]), "bq": f32(inputs["bq"]),
            "Wk": f32(inputs["Wk"]), "bk": f32(inputs["bk"]),
            "Wv": f32(inputs["Wv"]), "bv": f32(inputs["bv"]),
            "Wo": f32(inputs["Wo"]), "bo": f32(inputs["bo"]),
        })
    return maps


def run(inputs, trace=False):
    from concourse.bass_utils import run_bass_kernel_spmd
    if "nc" not in _CACHE:
        _CACHE["nc"] = _build()
    res = run_bass_kernel_spmd(
        _CACHE["nc"], _in_maps(inputs), core_ids=list(range(NCORES)), trace=trace,
    )
    outs = res.results
    attention_output = np.concatenate([np.asarray(r["out"]) for r in outs], axis=0)
    attention_probs = np.concatenate([np.asarray(r["probs"]) for r in outs], axis=0)
    contribution = np.concatenate([np.asarray(r["contrib"]) for r in outs], axis=0)
    return (attention_output, attention_probs, contribution), res


def kernel(**inputs):
    (attention_output, attention_probs, contribution), _ = run(inputs, trace=False)
    return attention_output, attention_probs, contribution


# revision 13
# speedup vs baseline: 1.0351x; 1.0351x over previous
"""Trainium2 Bass kernel for the custom attention module.

Self-contained: hardcodes shapes B=16, S=626, D=768, H=12, HD=64.
Shards batch over 8 NeuronCores (2 batches/core), no collectives.

Outputs (matching the reference): (attention_output, attention_probs, contribution)
"""

import numpy as np

B, S, D, H, HD = 16, 626, 768, 12, 64
NCORES = 8
BL = B // NCORES  # batches per core

# token tiles of 626: (start, size)
QTS = [(0, 128), (128, 128), (256, 128), (384, 128), (512, 114)]
N626 = [(0, 512), (512, 114)]   # free-dim chunks for 626-wide matmul outputs
N768 = [(0, 512), (512, 256)]   # free-dim chunks for 768-wide matmul outputs

_CACHE = {}


def _build():
    import concourse.bass as bass
    import concourse.mybir as mybir
    from concourse import bacc
    from concourse.tile import TileContext
    from concourse.masks import make_identity
    from concourse.bass import ds
    from contextlib import ExitStack

    F32 = mybir.dt.float32
    BF = mybir.dt.bfloat16
    I32 = mybir.dt.int32
    AX = mybir.AxisListType.X
    ALU = mybir.AluOpType
    AF = mybir.ActivationFunctionType

    nc = bacc.Bacc()

    hid = nc.declare_dram_parameter("hidden_states", [BL, S, D], F32, isOutput=False)
    msk = nc.declare_dram_parameter("mask", [BL, S - 1], I32, isOutput=False)
    wq_d = nc.declare_dram_parameter("Wq", [D, D], F32, isOutput=False)
    bq_d = nc.declare_dram_parameter("bq", [D], F32, isOutput=False)
    wk_d = nc.declare_dram_parameter("Wk", [D, D], F32, isOutput=False)
    bk_d = nc.declare_dram_parameter("bk", [D], F32, isOutput=False)
    wv_d = nc.declare_dram_parameter("Wv", [D, D], F32, isOutput=False)
    bv_d = nc.declare_dram_parameter("bv", [D], F32, isOutput=False)
    wo_d = nc.declare_dram_parameter("Wo", [D, D], F32, isOutput=False)
    bo_d = nc.declare_dram_parameter("bo", [D], F32, isOutput=False)

    o_out = nc.declare_dram_parameter("out", [BL, S, D], F32, isOutput=True)
    o_prb = nc.declare_dram_parameter("probs", [BL, H, S, S], F32, isOutput=True)
    o_ctr = nc.declare_dram_parameter("contrib", [BL, H, S], F32, isOutput=True)

    with ExitStack() as ctx:
        tc = ctx.enter_context(TileContext(nc))
        singles = ctx.enter_context(tc.tile_pool(name="singles", bufs=1))
        pool = ctx.enter_context(tc.tile_pool(name="work", bufs=2))
        pp_mm = ctx.enter_context(tc.tile_pool(name="psmm", bufs=3, space="PSUM"))
        pp_tp = ctx.enter_context(tc.tile_pool(name="pstp", bufs=2, space="PSUM"))

        # ---------- one-time setup ----------
        ident = singles.tile([128, 128], BF, tag="ident")
        make_identity(nc, ident)
        ones_bf = singles.tile([1, 128], BF, tag="ones")
        nc.vector.memset(ones_bf, 1.0)

        # weights, cast to bf16 during DMA (SWDGE).  lhsT layout [c=d_in, d_out]
        w_sb = {}
        for name, wd in (("q", wq_d), ("k", wk_d), ("v", wv_d), ("o", wo_d)):
            w_sb[name] = []
            for j in range(6):
                t = singles.tile([128, D], BF, tag=f"w{name}{j}")
                nc.gpsimd.dma_start(out=t, in_=wd[ds(j * 128, 128), :])
                w_sb[name].append(t)

        # per-partition bias chunks for QT/KT copyback (ACT bias operand, f32)
        bq_sb, bk_sb = [], []
        for bname, bd, lst in (("bq", bq_d, bq_sb), ("bk", bk_d, bk_sb)):
            bv_ap = bd[:].rearrange("(p o) -> p o", o=1)  # [768, 1]
            for j in range(6):
                t = singles.tile([128, 1], F32, tag=f"{bname}{j}")
                nc.sync.dma_start(out=t, in_=bv_ap[ds(j * 128, 128), :])
                lst.append(t)
        # bv as a bf16 row for the bias-row matmul trick
        bv_bf = singles.tile([1, D], BF, tag="bvbf")
        nc.gpsimd.dma_start(out=bv_bf, in_=bv_d[:].rearrange("(o d) -> o d", o=1))
        # bo broadcast to all 128 partitions (for the output epilogue add)
        bo_bc = singles.tile([128, D], F32, tag="bobc")
        bo_row = bo_d[:].rearrange("(o d) -> o d", o=1)  # [1, 768]
        nc.sync.dma_start(out=bo_bc, in_=bo_row.to_broadcast((128, D)))

        # ---------- per-batch ----------
        for b in range(BL):
            # boost row: 0.25 where mask626 == 0 (col 0 is the prepended CLS zero)
            mask_sb = pool.tile([1, S - 1], I32, tag="msk")
            nc.sync.dma_start(out=mask_sb, in_=msk[b].rearrange("(o k) -> o k", o=1))
            boost = pool.tile([1, S], F32, tag="boost")
            nc.vector.memset(boost[0:1, 0:1], 0.25)
            nc.vector.tensor_scalar(
                out=boost[0:1, 1:S], in0=mask_sb, scalar1=0, scalar2=0.25,
                op0=ALU.is_equal, op1=ALU.mult,
            )

            # hidden load (cast to bf16) + PE transpose -> hT [768, 626]
            hid_bf = []
            for (ts_, tz) in QTS:
                t = pool.tile([128, D], BF, tag="hidbf", bufs=5)
                nc.gpsimd.dma_start(out=t[0:tz, :], in_=hid[b, ds(ts_, tz), :])
                hid_bf.append(t)
            hT = []
            for j in range(6):
                ps = pp_tp.tile([128, S], BF, tag="tp")
                for ti, (ts_, tz) in enumerate(QTS):
                    nc.tensor.transpose(
                        out=ps[0:128, ds(ts_, tz)],
                        in_=hid_bf[ti][0:tz, ds(j * 128, 128)],
                        identity=ident[0:tz, 0:tz],
                    )
                t = pool.tile([128, S], BF, tag="hT", bufs=6)
                nc.scalar.copy(out=t, in_=ps)
                hT.append(t)

            # QKV projections
            QT_sb, KT_sb = [], []
            for wname, blst, outlst in (("q", bq_sb, QT_sb), ("k", bk_sb, KT_sb)):
                for j in range(6):
                    ps = pp_mm.tile([128, S], F32, tag="mm")
                    for (ns, nz) in N626:
                        for c in range(6):
                            nc.tensor.matmul(
                                out=ps[:, ds(ns, nz)],
                                lhsT=w_sb[wname][c][:, ds(j * 128, 128)],
                                rhs=hT[c][:, ds(ns, nz)],
                                start=(c == 0), stop=(c == 5),
                            )
                    t = pool.tile([128, S], BF, tag=f"{wname}T", bufs=6)
                    nc.vector.tensor_scalar(
                        out=t, in0=ps, scalar1=blst[j][0:128, 0:1], scalar2=None,
                        op0=ALU.add,
                    )
                    outlst.append(t)
            V_sb = []
            for (ts_, tz) in QTS:
                ps = pp_mm.tile([128, D], F32, tag="mm")
                for (ns, nz) in N768:
                    for c in range(6):
                        nc.tensor.matmul(
                            out=ps[0:tz, ds(ns, nz)],
                            lhsT=hT[c][:, ds(ts_, tz)],
                            rhs=w_sb["v"][c][:, ds(ns, nz)],
                            start=(c == 0), stop=False,
                        )
                    nc.tensor.matmul(
                        out=ps[0:tz, ds(ns, nz)],
                        lhsT=ones_bf[0:1, 0:tz],
                        rhs=bv_bf[0:1, ds(ns, nz)],
                        start=False, stop=True,
                    )
                t = pool.tile([128, D], BF, tag="V", bufs=5)
                nc.vector.tensor_copy(out=t[0:tz, :], in_=ps[0:tz, :])
                V_sb.append(t)

            craw = pool.tile([H, S], F32, tag="craw")
            ctxT_sb = []

            # ---------- head pairs ----------
            for p in range(H // 2):
                rmax = {}
                c0row = {}
                # col0 mini-matmuls: scores[:, 0] as a row over q (for contribution)
                for m in (0, 1):
                    h = 2 * p + m
                    c0 = pp_mm.tile([1, S], F32, tag="mm")
                    for (ns, nz) in N626:
                        nc.tensor.matmul(
                            out=c0[0:1, ds(ns, nz)],
                            lhsT=KT_sb[p][ds(64 * m, 64), 0:1],
                            rhs=QT_sb[p][ds(64 * m, 64), ds(ns, nz)],
                            start=True, stop=True,
                        )
                    t0 = pool.tile([1, S], F32, tag="c0row", bufs=3)
                    nc.scalar.copy(out=t0, in_=c0[0:1, :])
                    c0row[m] = t0

                norm_sb = {}
                for m in (0, 1):
                    nrm = pool.tile([128, 5 * S], BF, tag=f"norm{m}", name=f"nrm{m}")
                    norm_sb[m] = nrm
                # interleave heads A/B so their K=64 matmuls land in different
                # PE row groups (tile_position from base_partition) and overlap
                for qi, (qs, qz) in enumerate(QTS):
                    for m in (0, 1):
                        h = 2 * p + m
                        nrm = norm_sb[m]
                        sps = pp_mm.tile([128, S], F32, tag="mm")
                        for (ns, nz) in N626:
                            nc.tensor.matmul(
                                out=sps[0:qz, ds(ns, nz)],
                                lhsT=QT_sb[p][ds(64 * m, 64), ds(qs, qz)],
                                rhs=KT_sb[p][ds(64 * m, 64), ds(ns, nz)],
                                start=True, stop=True,
                            )
                        r0c = None
                        if qi == 0:
                            # CLS-row boost, done on an SBUF copy so that DVE
                            # never reads the scores PSUM (keeps matmul waits <= 2)
                            r0c = pool.tile([1, S], F32, tag="r0c", bufs=2)
                            nc.scalar.copy(out=r0c, in_=sps[0:1, :])
                            rx = pool.tile([1, 1], F32, tag="rmax", bufs=4)
                            nc.vector.reduce_max(out=rx, in_=r0c, axis=AX)
                            rmax[m] = rx
                            btmp = pool.tile([1, S], F32, tag="btmp")
                            nc.vector.tensor_scalar(
                                out=btmp, in0=boost, scalar1=rx[0:1, 0:1],
                                scalar2=None, op0=ALU.mult,
                            )
                            nc.vector.tensor_tensor(
                                out=r0c, in0=r0c, in1=btmp, op=ALU.add,
                            )
                            # contribution element q=0 gets the same boost (mask626[0]=0)
                            ctmp = pool.tile([1, 1], F32, tag="ctmp")
                            nc.vector.tensor_scalar(
                                out=ctmp, in0=rx, scalar1=0.25, scalar2=None,
                                op0=ALU.mult,
                            )
                            nc.vector.tensor_tensor(
                                out=c0row[m][0:1, 0:1], in0=c0row[m][0:1, 0:1],
                                in1=ctmp[0:1, 0:1], op=ALU.add,
                            )
                            # assemble the fixed row into the packed per-head table
                            nc.sync.dma_start(
                                out=craw[h:h + 1, :], in_=c0row[m],
                            )
                        ex = pool.tile([128, S], BF, tag="exp", bufs=6)
                        sm = pool.tile([128, 1], F32, tag="sums", bufs=8)
                        nc.scalar.activation(
                            out=ex[0:qz, :], in_=sps[0:qz, :], func=AF.Exp,
                            bias=0.0, scale=0.125, accum_out=sm[0:qz, 0:1],
                        )
                        if qi == 0:
                            # overwrite row 0 with the boosted version (+ its sum)
                            nc.scalar.activation(
                                out=ex[0:1, :], in_=r0c, func=AF.Exp,
                                bias=0.0, scale=0.125, accum_out=sm[0:1, 0:1],
                            )
                        rc = pool.tile([128, 1], F32, tag="rcp", bufs=8)
                        nc.vector.reciprocal(out=rc[0:qz, :], in_=sm[0:qz, :])
                        nc.vector.tensor_scalar(
                            out=nrm[0:qz, ds(qi * S, S)], in0=ex[0:qz, :],
                            scalar1=rc[0:qz, 0:1], scalar2=None, op0=ALU.mult,
                        )
                # fp32 probs output via casting SWDGE DMA (bf16 -> f32)
                for m in (0, 1):
                    h = 2 * p + m
                    nrm = norm_sb[m]
                    nc.gpsimd.dma_start(
                        out=o_prb[b, h, 0:512, :].rearrange("(t p) k -> p t k", p=128),
                        in_=nrm[0:128, 0:4 * S].rearrange("p (t k) -> p t k", k=S),
                    )
                    nc.gpsimd.dma_start(
                        out=o_prb[b, h, 512:S, :],
                        in_=nrm[0:114, ds(4 * S, S)],
                    )

                # transpose probs + context matmul (accumulated over k tiles)
                cps = pp_mm.tile([128, S], F32, tag="mm")
                for ki, (ks, kz) in enumerate(QTS):
                    for m in (0, 1):
                        h = 2 * p + m
                        eps = pp_tp.tile([128, S], BF, tag="tp")
                        for qi, (qs, qz) in enumerate(QTS):
                            nc.tensor.transpose(
                                out=eps[0:kz, ds(qs, qz)],
                                in_=norm_sb[m][0:qz, ds(qi * S + ks, kz)],
                                identity=ident[0:qz, 0:qz],
                            )
                        eT = pool.tile([128, S], BF, tag="eT", bufs=3)
                        nc.vector.tensor_copy(out=eT[0:kz, :], in_=eps[0:kz, :])
                        for (ns, nz) in N626:
                            nc.tensor.matmul(
                                out=cps[ds(64 * m, 64), ds(ns, nz)],
                                lhsT=V_sb[ki][0:kz, ds(64 * h, 64)],
                                rhs=eT[0:kz, ds(ns, nz)],
                                start=(ki == 0), stop=(ki == 4),
                            )
                ct = pool.tile([128, S], BF, tag="ctxT", bufs=6)
                nc.vector.tensor_copy(out=ct, in_=cps)
                ctxT_sb.append(ct)

            # contribution: softmax over q of scores[:, 0] per head
            ce = pool.tile([H, S], F32, tag="ce")
            cs = pool.tile([H, 1], F32, tag="cs")
            nc.scalar.activation(
                out=ce, in_=craw, func=AF.Exp, bias=0.0, scale=0.125,
                accum_out=cs[0:H, 0:1],
            )
            cr = pool.tile([H, 1], F32, tag="cr")
            nc.vector.reciprocal(out=cr, in_=cs)
            co = pool.tile([H, S], F32, tag="co")
            nc.vector.tensor_scalar(
                out=co, in0=ce, scalar1=cr[0:H, 0:1], scalar2=None, op0=ALU.mult,
            )
            nc.sync.dma_start(out=o_ctr[b], in_=co)

            # output projection + bias
            for (qs, qz) in QTS:
                ops = pp_mm.tile([128, D], F32, tag="mm")
                for (ns, nz) in N768:
                    for c in range(6):
                        nc.tensor.matmul(
                            out=ops[0:qz, ds(ns, nz)],
                            lhsT=ctxT_sb[c][:, ds(qs, qz)],
                            rhs=w_sb["o"][c][:, ds(ns, nz)],
                            start=(c == 0), stop=(c == 5),
                        )
                osb = pool.tile([128, D], F32, tag="osb")
                nc.vector.tensor_tensor(
                    out=osb[0:qz, :], in0=ops[0:qz, :], in1=bo_bc[0:qz, :],
                    op=ALU.add,
                )
                nc.sync.dma_start(out=o_out[b, ds(qs, qz), :], in_=osb[0:qz, :])

    nc.compile()
    return nc


def _in_maps(inputs):
    f32 = lambda x: np.ascontiguousarray(np.asarray(x), dtype=np.float32)
    maps = []
    for i in range(NCORES):
        sl = slice(BL * i, BL * (i + 1))
        maps.append({
            "hidden_states": f32(inputs["hidden_states"][sl]),
            "mask": np.ascontiguousarray(np.asarray(inputs["mask"][sl]), dtype=np.int32),
            "Wq": f32(inputs["Wq"]), "bq": f32(inputs["bq"]),
            "Wk": f32(inputs["Wk"]), "bk": f32(inputs["bk"]),
            "Wv": f32(inputs["Wv"]), "bv": f32(inputs["bv"]),
            "Wo": f32(inputs["Wo"]), "bo": f32(inputs["bo"]),
        })
    return maps


def run(inputs, trace=False):
    from concourse.bass_utils import run_bass_kernel_spmd
    if "nc" not in _CACHE:
        _CACHE["nc"] = _build()
    res = run_bass_kernel_spmd(
        _CACHE["nc"], _in_maps(inputs), core_ids=list(range(NCORES)), trace=trace,
    )
    outs = res.results
    attention_output = np.concatenate([np.asarray(r["out"]) for r in outs], axis=0)
    attention_probs = np.concatenate([np.asarray(r["probs"]) for r in outs], axis=0)
    contribution = np.concatenate([np.asarray(r["contrib"]) for r in outs], axis=0)
    return (attention_output, attention_probs, contribution), res


def kernel(**inputs):
    (attention_output, attention_probs, contribution), _ = run(inputs, trace=False)
    return attention_output, attention_probs, contribution


# revision 17
# speedup vs baseline: 1.1089x; 1.0714x over previous
"""Trainium2 Bass kernel for the custom attention module.

Self-contained: hardcodes shapes B=16, S=626, D=768, H=12, HD=64.
Shards batch over 8 NeuronCores (2 batches/core), no collectives.

Outputs (matching the reference): (attention_output, attention_probs, contribution)
"""

import numpy as np

B, S, D, H, HD = 16, 626, 768, 12, 64
NCORES = 8
BL = B // NCORES  # batches per core

# token tiles of 626: (start, size)
QTS = [(0, 128), (128, 128), (256, 128), (384, 128), (512, 114)]
N626 = [(0, 512), (512, 114)]   # free-dim chunks for 626-wide matmul outputs
N768 = [(0, 512), (512, 256)]   # free-dim chunks for 768-wide matmul outputs

_CACHE = {}


def _build():
    import concourse.bass as bass
    import concourse.mybir as mybir
    from concourse import bacc
    from concourse.tile import TileContext
    from concourse.masks import make_identity
    from concourse.bass import ds
    from contextlib import ExitStack

    F32 = mybir.dt.float32
    BF = mybir.dt.bfloat16
    I32 = mybir.dt.int32
    AX = mybir.AxisListType.X
    ALU = mybir.AluOpType
    AF = mybir.ActivationFunctionType

    nc = bacc.Bacc()

    hid = nc.declare_dram_parameter("hidden_states", [BL, S, D], F32, isOutput=False)
    msk = nc.declare_dram_parameter("mask", [BL, S - 1], I32, isOutput=False)
    wq_d = nc.declare_dram_parameter("Wq", [D, D], F32, isOutput=False)
    bq_d = nc.declare_dram_parameter("bq", [D], F32, isOutput=False)
    wk_d = nc.declare_dram_parameter("Wk", [D, D], F32, isOutput=False)
    bk_d = nc.declare_dram_parameter("bk", [D], F32, isOutput=False)
    wv_d = nc.declare_dram_parameter("Wv", [D, D], F32, isOutput=False)
    bv_d = nc.declare_dram_parameter("bv", [D], F32, isOutput=False)
    wo_d = nc.declare_dram_parameter("Wo", [D, D], F32, isOutput=False)
    bo_d = nc.declare_dram_parameter("bo", [D], F32, isOutput=False)

    o_out = nc.declare_dram_parameter("out", [BL, S, D], F32, isOutput=True)
    o_prb = nc.declare_dram_parameter("probs", [BL, H, S, S], F32, isOutput=True)
    o_ctr = nc.declare_dram_parameter("contrib", [BL, H, S], F32, isOutput=True)

    with ExitStack() as ctx:
        tc = ctx.enter_context(TileContext(nc))
        singles = ctx.enter_context(tc.tile_pool(name="singles", bufs=1))
        pool = ctx.enter_context(tc.tile_pool(name="work", bufs=2))
        pp_mm = ctx.enter_context(tc.tile_pool(name="psmm", bufs=3, space="PSUM"))
        pp_tp = ctx.enter_context(tc.tile_pool(name="pstp", bufs=2, space="PSUM"))

        # ---------- one-time setup ----------
        ident = singles.tile([128, 128], BF, tag="ident")
        make_identity(nc, ident)
        ones_bf = singles.tile([1, 128], BF, tag="ones")
        nc.vector.memset(ones_bf, 1.0)

        # hidden for batch 0 first — unblocks the PE transposes while the
        # (much larger) weight loads stream in behind it on the same queue
        hid_bf_all = {}
        for b in range(BL):
            hid_bf_all[b] = []
        for (ts_, tz) in QTS:
            t = pool.tile([128, D], BF, tag="hidbf", bufs=8, name="hb0")
            nc.gpsimd.dma_start(out=t[0:tz, :], in_=hid[0, ds(ts_, tz), :])
            hid_bf_all[0].append(t)

        # weights, cast to bf16 during DMA (SWDGE).  lhsT layout [c=d_in, d_out]
        w_sb = {}
        for name, wd in (("q", wq_d), ("k", wk_d), ("v", wv_d), ("o", wo_d)):
            w_sb[name] = []
            for j in range(6):
                t = singles.tile([128, D], BF, tag=f"w{name}{j}")
                nc.gpsimd.dma_start(out=t, in_=wd[ds(j * 128, 128), :])
                w_sb[name].append(t)

        # per-partition bias chunks for QT/KT copyback (ACT bias operand, f32)
        bq_sb, bk_sb = [], []
        for bname, bd, lst in (("bq", bq_d, bq_sb), ("bk", bk_d, bk_sb)):
            bv_ap = bd[:].rearrange("(p o) -> p o", o=1)  # [768, 1]
            for j in range(6):
                t = singles.tile([128, 1], F32, tag=f"{bname}{j}")
                nc.sync.dma_start(out=t, in_=bv_ap[ds(j * 128, 128), :])
                lst.append(t)
        # bv as a bf16 row for the bias-row matmul trick
        bv_bf = singles.tile([1, D], BF, tag="bvbf")
        nc.gpsimd.dma_start(out=bv_bf, in_=bv_d[:].rearrange("(o d) -> o d", o=1))
        # bo broadcast to all 128 partitions (for the output epilogue add)
        bo_bc = singles.tile([128, D], F32, tag="bobc")
        bo_row = bo_d[:].rearrange("(o d) -> o d", o=1)  # [1, 768]
        nc.sync.dma_start(out=bo_bc, in_=bo_row.to_broadcast((128, D)))

        # ---------- per-batch ----------
        for b in range(BL):
            # boost row: 0.25 where mask626 == 0 (col 0 is the prepended CLS zero)
            mask_sb = pool.tile([1, S - 1], I32, tag="msk")
            nc.sync.dma_start(out=mask_sb, in_=msk[b].rearrange("(o k) -> o k", o=1))
            boost = pool.tile([1, S], F32, tag="boost")
            nc.vector.memset(boost[0:1, 0:1], 0.25)
            nc.vector.tensor_scalar(
                out=boost[0:1, 1:S], in0=mask_sb, scalar1=0, scalar2=0.25,
                op0=ALU.is_equal, op1=ALU.mult,
            )

            # hidden load (cast to bf16) + PE transpose -> hT [768, 626]
            hid_bf = hid_bf_all[b]
            if not hid_bf:
                for (ts_, tz) in QTS:
                    t = pool.tile([128, D], BF, tag="hidbf", bufs=8)
                    nc.gpsimd.dma_start(out=t[0:tz, :], in_=hid[b, ds(ts_, tz), :])
                    hid_bf.append(t)
            hT = []
            for j in range(6):
                ps = pp_tp.tile([128, S], BF, tag="tp")
                for ti, (ts_, tz) in enumerate(QTS):
                    nc.tensor.transpose(
                        out=ps[0:128, ds(ts_, tz)],
                        in_=hid_bf[ti][0:tz, ds(j * 128, 128)],
                        identity=ident[0:tz, 0:tz],
                    )
                t = pool.tile([128, S], BF, tag="hT", bufs=8)
                nc.scalar.copy(out=t, in_=ps)
                hT.append(t)

            # QKV projections
            QT_sb, KT_sb = [], []
            for wname, blst, outlst in (("q", bq_sb, QT_sb), ("k", bk_sb, KT_sb)):
                for j in range(6):
                    ps = pp_mm.tile([128, S], F32, tag="mm")
                    for (ns, nz) in N626:
                        for c in range(6):
                            nc.tensor.matmul(
                                out=ps[:, ds(ns, nz)],
                                lhsT=w_sb[wname][c][:, ds(j * 128, 128)],
                                rhs=hT[c][:, ds(ns, nz)],
                                start=(c == 0), stop=(c == 5),
                            )
                    t = pool.tile([128, S], BF, tag=f"{wname}T", bufs=12)
                    nc.vector.tensor_scalar(
                        out=t, in0=ps, scalar1=blst[j][0:128, 0:1], scalar2=None,
                        op0=ALU.add,
                    )
                    outlst.append(t)
            V_sb = []
            for (ts_, tz) in QTS:
                ps = pp_mm.tile([128, D], F32, tag="mm")
                for (ns, nz) in N768:
                    for c in range(6):
                        nc.tensor.matmul(
                            out=ps[0:tz, ds(ns, nz)],
                            lhsT=hT[c][:, ds(ts_, tz)],
                            rhs=w_sb["v"][c][:, ds(ns, nz)],
                            start=(c == 0), stop=False,
                        )
                    nc.tensor.matmul(
                        out=ps[0:tz, ds(ns, nz)],
                        lhsT=ones_bf[0:1, 0:tz],
                        rhs=bv_bf[0:1, ds(ns, nz)],
                        start=False, stop=True,
                    )
                t = pool.tile([128, D], BF, tag="V", bufs=10)
                nc.vector.tensor_copy(out=t[0:tz, :], in_=ps[0:tz, :])
                V_sb.append(t)

            craw = pool.tile([H, S], F32, tag="craw")
            ctxT_sb = []

            # ---------- head pairs ----------
            for p in range(H // 2):
                rmax = {}
                c0row = {}
                # col0 mini-matmuls: scores[:, 0] as a row over q (for contribution)
                for m in (0, 1):
                    h = 2 * p + m
                    c0 = pp_mm.tile([1, S], F32, tag="mm")
                    for (ns, nz) in N626:
                        nc.tensor.matmul(
                            out=c0[0:1, ds(ns, nz)],
                            lhsT=KT_sb[p][ds(64 * m, 64), 0:1],
                            rhs=QT_sb[p][ds(64 * m, 64), ds(ns, nz)],
                            start=True, stop=True,
                        )
                    t0 = pool.tile([1, S], F32, tag="c0row", bufs=3)
                    nc.scalar.copy(out=t0, in_=c0[0:1, :])
                    c0row[m] = t0

                norm_sb = {}
                for m in (0, 1):
                    nrm = pool.tile([128, 5 * S], BF, tag=f"norm{m}", name=f"nrm{m}")
                    norm_sb[m] = nrm
                # interleave heads A/B so their K=64 matmuls land in different
                # PE row groups (tile_position from base_partition) and overlap
                for qi, (qs, qz) in enumerate(QTS):
                    for m in (0, 1):
                        h = 2 * p + m
                        nrm = norm_sb[m]
                        sps = pp_mm.tile([128, S], F32, tag="mm")
                        for (ns, nz) in N626:
                            nc.tensor.matmul(
                                out=sps[0:qz, ds(ns, nz)],
                                lhsT=QT_sb[p][ds(64 * m, 64), ds(qs, qz)],
                                rhs=KT_sb[p][ds(64 * m, 64), ds(ns, nz)],
                                start=True, stop=True,
                            )
                        r0c = None
                        if qi == 0:
                            # CLS-row boost, done on an SBUF copy so that DVE
                            # never reads the scores PSUM (keeps matmul waits <= 2)
                            r0c = pool.tile([1, S], F32, tag="r0c", bufs=2)
                            nc.scalar.copy(out=r0c, in_=sps[0:1, :])
                            rx = pool.tile([1, 1], F32, tag="rmax", bufs=4)
                            nc.vector.reduce_max(out=rx, in_=r0c, axis=AX)
                            rmax[m] = rx
                            btmp = pool.tile([1, S], F32, tag="btmp")
                            nc.vector.tensor_scalar(
                                out=btmp, in0=boost, scalar1=rx[0:1, 0:1],
                                scalar2=None, op0=ALU.mult,
                            )
                            nc.vector.tensor_tensor(
                                out=r0c, in0=r0c, in1=btmp, op=ALU.add,
                            )
                            # contribution element q=0 gets the same boost (mask626[0]=0)
                            ctmp = pool.tile([1, 1], F32, tag="ctmp")
                            nc.vector.tensor_scalar(
                                out=ctmp, in0=rx, scalar1=0.25, scalar2=None,
                                op0=ALU.mult,
                            )
                            nc.vector.tensor_tensor(
                                out=c0row[m][0:1, 0:1], in0=c0row[m][0:1, 0:1],
                                in1=ctmp[0:1, 0:1], op=ALU.add,
                            )
                            # assemble the fixed row into the packed per-head table
                            nc.gpsimd.dma_start(
                                out=craw[h:h + 1, :], in_=c0row[m],
                            )
                        ex = pool.tile([128, S], BF, tag="exp", bufs=5)
                        sm = pool.tile([128, 1], F32, tag="sums", bufs=8)
                        nc.scalar.activation(
                            out=ex[0:qz, :], in_=sps[0:qz, :], func=AF.Exp,
                            bias=0.0, scale=0.125, accum_out=sm[0:qz, 0:1],
                        )
                        if qi == 0:
                            # overwrite row 0 with the boosted version (+ its sum)
                            nc.scalar.activation(
                                out=ex[0:1, :], in_=r0c, func=AF.Exp,
                                bias=0.0, scale=0.125, accum_out=sm[0:1, 0:1],
                            )
                        rc = pool.tile([128, 1], F32, tag="rcp", bufs=8)
                        nc.vector.reciprocal(out=rc[0:qz, :], in_=sm[0:qz, :])
                        nc.vector.tensor_scalar(
                            out=nrm[0:qz, ds(qi * S, S)], in0=ex[0:qz, :],
                            scalar1=rc[0:qz, 0:1], scalar2=None, op0=ALU.mult,
                        )
                # fp32 probs output via casting SWDGE DMA (bf16 -> f32)
                for m in (0, 1):
                    h = 2 * p + m
                    nrm = norm_sb[m]
                    nc.gpsimd.dma_start(
                        out=o_prb[b, h, 0:512, :].rearrange("(t p) k -> p t k", p=128),
                        in_=nrm[0:128, 0:4 * S].rearrange("p (t k) -> p t k", k=S),
                    )
                    nc.gpsimd.dma_start(
                        out=o_prb[b, h, 512:S, :],
                        in_=nrm[0:114, ds(4 * S, S)],
                    )

                # transpose probs + context matmul (accumulated over k tiles)
                cps = pp_mm.tile([128, S], F32, tag="mm")
                for ki, (ks, kz) in enumerate(QTS):
                    for m in (0, 1):
                        h = 2 * p + m
                        eps = pp_tp.tile([128, S], BF, tag="tp")
                        for qi, (qs, qz) in enumerate(QTS):
                            nc.tensor.transpose(
                                out=eps[0:kz, ds(qs, qz)],
                                in_=norm_sb[m][0:qz, ds(qi * S + ks, kz)],
                                identity=ident[0:qz, 0:qz],
                            )
                        eT = pool.tile([128, S], BF, tag="eT", bufs=3)
                        nc.vector.tensor_copy(out=eT[0:kz, :], in_=eps[0:kz, :])
                        for (ns, nz) in N626:
                            nc.tensor.matmul(
                                out=cps[ds(64 * m, 64), ds(ns, nz)],
                                lhsT=V_sb[ki][0:kz, ds(64 * h, 64)],
                                rhs=eT[0:kz, ds(ns, nz)],
                                start=(ki == 0), stop=(ki == 4),
                            )
                ct = pool.tile([128, S], BF, tag="ctxT", bufs=12)
                nc.vector.tensor_copy(out=ct, in_=cps)
                ctxT_sb.append(ct)

            # contribution: softmax over q of scores[:, 0] per head
            ce = pool.tile([H, S], F32, tag="ce")
            cs = pool.tile([H, 1], F32, tag="cs")
            nc.scalar.activation(
                out=ce, in_=craw, func=AF.Exp, bias=0.0, scale=0.125,
                accum_out=cs[0:H, 0:1],
            )
            cr = pool.tile([H, 1], F32, tag="cr")
            nc.vector.reciprocal(out=cr, in_=cs)
            co = pool.tile([H, S], F32, tag="co")
            nc.vector.tensor_scalar(
                out=co, in0=ce, scalar1=cr[0:H, 0:1], scalar2=None, op0=ALU.mult,
            )
            nc.sync.dma_start(out=o_ctr[b], in_=co)

            # output projection + bias
            for (qs, qz) in QTS:
                ops = pp_mm.tile([128, D], F32, tag="mm")
                for (ns, nz) in N768:
                    for c in range(6):
                        nc.tensor.matmul(
                            out=ops[0:qz, ds(ns, nz)],
                            lhsT=ctxT_sb[c][:, ds(qs, qz)],
                            rhs=w_sb["o"][c][:, ds(ns, nz)],
                            start=(c == 0), stop=(c == 5),
                        )
                osb = pool.tile([128, D], F32, tag="osb")
                nc.vector.tensor_tensor(
                    out=osb[0:qz, :], in0=ops[0:qz, :], in1=bo_bc[0:qz, :],
                    op=ALU.add,
                )
                nc.sync.dma_start(out=o_out[b, ds(qs, qz), :], in_=osb[0:qz, :])

    nc.compile()
    return nc


def _in_maps(inputs):
    f32 = lambda x: np.ascontiguousarray(np.asarray(x), dtype=np.float32)
    maps = []
    for i in range(NCORES):
        sl = slice(BL * i, BL * (i + 1))
        maps.append({
            "hidden_states": f32(inputs["hidden_states"][sl]),
            "mask": np.ascontiguousarray(np.asarray(inputs["mask"][sl]), dtype=np.int32),
            "Wq": f32(inputs["Wq"]), "bq": f32(inputs["bq"]),
            "Wk": f32(inputs["Wk"]), "bk": f32(inputs["bk"]),
            "Wv": f32(inputs["Wv"]), "bv": f32(inputs["bv"]),
            "Wo": f32(inputs["Wo"]), "bo": f32(inputs["bo"]),
        })
    return maps


def run(inputs, trace=False):
    from concourse.bass_utils import run_bass_kernel_spmd
    if "nc" not in _CACHE:
        _CACHE["nc"] = _build()
    res = run_bass_kernel_spmd(
        _CACHE["nc"], _in_maps(inputs), core_ids=list(range(NCORES)), trace=trace,
    )
    outs = res.results
    attention_output = np.concatenate([np.asarray(r["out"]) for r in outs], axis=0)
    attention_probs = np.concatenate([np.asarray(r["probs"]) for r in outs], axis=0)
    contribution = np.concatenate([np.asarray(r["contrib"]) for r in outs], axis=0)
    return (attention_output, attention_probs, contribution), res


def kernel(**inputs):
    (attention_output, attention_probs, contribution), _ = run(inputs, trace=False)
    return attention_output, attention_probs, contribution
